# revision 19
# baseline (speedup 1.0000x reference)
"""Multi-head attention (B=8, N=1024, D=768, H=12) on 8 TRN2 NeuronCores.

Sharding: data-parallel over batch - core b computes batch element b.

Host-prepped per-core inputs (QK-path operands shipped as fp16 - the
e5m10 mantissa keeps the end-to-end error ~6.4e-3 vs the 2e-2 budget,
halves DMA bytes, and 2-byte LDWEIGHTS double-buffer in the PE, unlike
fp32 weight loads which expose ~100-190ns per matmul):
    xa    (6, 128, 512)   x[b]^T contraction chunks, columns 0:512
    xb    (2, 128, 1536)  columns 512:1024, packed per-partition-
                          contiguous in halves for 3KB DMA descriptors
    wp    (6, 128, 1536)  W_q/W_k packed per head pair ([q cols | k cols]
                          per k-chunk; one contiguous DMA per pair)
    wv    (128, 4608)     W_v chunks packed per-partition-contiguous
    b_qk (128, 12), b_v (1, 768), ones_in (1, 128)
  output: out (12, 8, 128, 64) f32, host-reassembled to (1024, 768)

Entry: the first-S critical set (wp0 + xa, 1.18MB) is spread evenly
over the three DMA-capable queues (sync/scalar/gpsimd, ~21GB/s each).

Single fully-interleaved pipeline; ScalarE's exp stream (96 x [128,1024]
at ~1.11us each) is the near-critical resource next to TensorE:
  - qkT chunks ((x @ W_qk)^T -> fp16, recycled 8-buf pool) are produced
    one head pair ahead; the first pair computes only the n=0 column
    halves up front so the first exp fires ~7us earlier, with the n=1
    halves and their retroactive S slabs emitted as j=0..2 fillers.
  - v = x @ W_v (+ b_v via a ones-row matmul, skipped when bias is zero)
    is stored per head as bf16 [v 64 | ones 1] blocks (the ones column
    yields the softmax denominator inside the PV matmul).
  - S^T[j,i] = k @ q^T per head pair via row-tiled K=64 fp16 matmuls; the
    two heads share each [128,1024] PSUM slab (A left bank, B right) so
    one exp releases both heads' next matmuls and the pair runs
    concurrently on PE row groups 0:64 / 64:128.
  - E = exp(S^T) -> bf16 on ScalarE straight from PSUM (bf16, not fp16:
    exp of +-40 logits overflows fp16). No max subtraction: softmax is
    shift-invariant and the logits stay in fp32/bf16 exp range.
  - PV: psum[i-block, 0:65] = sum_j E^T[j] @ v_block; pv accumulation
    groups pack 4-per-PSUM-bank (128-elem slots of one [128,512] tile)
    so the pv chain never stalls on pool rotation. PV i-steps of the
    previous pair and next pair's qkT chunks fill PE slack between S
    j-steps - but never at j=0/j=7, where they would sit between
    adjacent pairs' S matmuls in the in-order PE queue and stall the
    exp stream across the pair boundary.
  - epilogue per (head, i): out = pv * recip(denom), all-DVE (a ScalarE
    hand-off would add a cross-engine sem hop per step, costly in the
    half-clock tail), then one 32KB DMA per (head, i-block) into
    out (H, MI, 128, 64); the host reassembles to (1024, 768) for free.
  - PSUM: 2 S-slab bufs (4 banks) ping-pong against the exp stream;
    4 shared mix bufs (4 banks) decouple the qk/v/pv psum rotations
    (with only 2, a qk chunk or pv chain regularly stalled ~1us waiting
    a pv tile's DVE drain, surfacing as exp-stream gaps at pair
    boundaries).

A dummy exp at kernel start pulls the ~1.4us ACT table load into the
idle entry phase.

The tail (the final pair's 16 PV steps, which cannot overlap their own
exps and always run at the HAM's half clock) fills all 16 PSUM groups
first - they fit the 4-buf mix pool exactly - then drains with one
strided reciprocal per 4-slot tile, minimizing serialized DVE ops.

Measured: ~159us HW exec per core (all 8 cores run the same program),
max-abs relative error 6.4e-3 vs the fp32 reference. Note the device
has two clock states; slow-state runs measure ~20% longer with ScalarE
ops uniformly slower (exp 1.34us vs 1.11us) - compare kernels only
across same-state runs (exp avg duration in the trace identifies the
state).
"""

import time
from collections import deque

import numpy as np

import concourse.mybir as mybir
import concourse.tile as tile
from concourse import bacc
from concourse.bass_utils import run_bass_kernel_spmd

N_CORES = 8
NSEQ = 1024
DMODEL = 768
H = 12
DH = 64
C3 = 3 * DMODEL
KC = DMODEL // 128   # 6 contraction chunks
MI = NSEQ // 128     # 8 sequence chunks
VB = DH + 1          # 65: per-head v block [v bf16 64 | ones 1]

F32 = mybir.dt.float32
F32R = mybir.dt.float32r
F16 = mybir.dt.float16
BF16 = mybir.dt.bfloat16
EXP = mybir.ActivationFunctionType.Exp
MUL = mybir.AluOpType.mult
ADD = mybir.AluOpType.add

_NC_CACHE = {}


def build_nc(with_bias=True):
    key = ("nc", with_bias)
    if key in _NC_CACHE:
        return _NC_CACHE[key]
    nc = bacc.Bacc("TRN2", target_bir_lowering=False, debug=False)
    xa_d = nc.dram_tensor("xa", [KC, 128, 512], F16, kind="ExternalInput")
    xb_d = nc.dram_tensor("xb", [2, 128, 3 * 512], F16, kind="ExternalInput")
    wp_d = nc.dram_tensor("wp", [KC, 128, KC * 256], F16, kind="ExternalInput")
    wv_d = nc.dram_tensor("wv", [128, KC * DMODEL], F16, kind="ExternalInput")
    bqk_d = nc.dram_tensor("b_qk", [128, 2 * KC], F32, kind="ExternalInput")
    bv_d = nc.dram_tensor("b_v", [1, DMODEL], F32R, kind="ExternalInput")
    ones_d = nc.dram_tensor("ones_in", [1, 128], F32R, kind="ExternalInput")
    out_d = nc.dram_tensor("out", [H, MI, 128, DH], F32, kind="ExternalOutput")

    with tile.TileContext(nc) as tc:
        with (
            tc.tile_pool(name="const", bufs=1) as cpool,
            tc.tile_pool(name="main", bufs=1) as mpool,
            tc.tile_pool(name="stage", bufs=12) as stpool,
            tc.tile_pool(name="e", bufs=34) as epool,
            tc.tile_pool(name="wt", bufs=3) as wpool,
            tc.tile_pool(name="qkt", bufs=8) as qkpool,
            tc.tile_pool(name="s_ps", bufs=2, space="PSUM") as sps,
            tc.tile_pool(name="mix_ps", bufs=4, space="PSUM") as mps,
        ):
            b_qk = cpool.tile([128, 2 * KC], F32, tag="bqk")
            nc.sync.dma_start(b_qk[:], bqk_d[:])
            warm = cpool.tile([128, 1], F32, tag="warm")
            nc.scalar.activation(warm[:], b_qk[:, 0:1], EXP)
            # PE warm-up: ~9us of dummy matmuls during the input-DMA wait
            # pulls the HAM to K=8/8 before the first real qk chunk, halving
            # the entry matmuls' duration (cold MMs run at 1.2 GHz).
            dummy = cpool.tile([128, 512], F16, tag="dummy")
            nc.vector.memset(dummy[:], 0.0)
            dps = mps.tile([128, 512], F32, tag="mps", name="ps_warm")
            NWARM = 14
            for r in range(NWARM):
                nc.tensor.matmul(
                    dps[:], lhsT=dummy[:, 0:128], rhs=dummy[:],
                    start=(r == 0), stop=(r == NWARM - 1),
                )

            # persistent activations
            v_ext = [mpool.tile([128, H * VB], BF16, tag=f"vx{j}", name=f"vx{j}")
                     for j in range(MI)]
            # x^T halves, k-chunks packed as column blocks of one tile
            xa_t = mpool.tile([128, KC * 512], F16, tag="xat", name="xat")
            xb_t = mpool.tile([128, KC * 512], F16, tag="xbt", name="xbt")

            # W_q/W_k packed per head pair: tile[:, k, 0:128] = q chunk cols,
            # tile[:, k, 128:256] = k chunk cols. One contiguous DMA per pair.
            w_t = {}

            def load_w(pm):
                t = wpool.tile([128, KC * 256], F16, tag="w", name=f"wp{pm}")
                nc.sync.dma_start(t[:], wp_d[pm])
                w_t[pm] = t

            # qkT chunks from a recycled pool (live: current + next pair)
            qkt = {}

            # inputs split across the three DMA-capable queues (sync/scalar/
            # gpsimd); the first-S critical set (wp0 + xa) leads each queue
            t0w = wpool.tile([128, KC * 256], F16, tag="w", name="wp0")
            nc.sync.dma_start(t0w[:, 0:768], wp_d[0][:, 0:768])
            nc.sync.dma_start(t0w[:, 768:1536], wp_d[0][:, 768:1536])
            w_t[0] = t0w
            # xa as two 3-chunk transfers, one per side queue
            xa3 = xa_t.rearrange("p (k c) -> p k c", c=512)
            nc.scalar.dma_start(xa3[:, 0:3], xa_d[0:3].rearrange("k p c -> p k c"))
            nc.gpsimd.dma_start(xa3[:, 3:6], xa_d[3:6].rearrange("k p c -> p k c"))

            with tc.tile_pool(name="wv", bufs=1) as wvpool:
                load_w(1)  # sync: wp1 ahead of the small const DMAs
                b_v = cpool.tile([1, DMODEL], F32R, tag="bv")
                nc.sync.dma_start(b_v[:], bv_d[:])
                ones1 = cpool.tile([1, 128], F32R, tag="ones")
                nc.sync.dma_start(ones1[:], ones_d[:])
                nc.scalar.dma_start(xb_t[:, 0:1536], xb_d[0])
                nc.gpsimd.dma_start(xb_t[:, 1536:3072], xb_d[1])
                wv_t = wvpool.tile([128, KC * DMODEL], F16, tag="wvt", name="wvt")
                nc.scalar.dma_start(wv_t[:, 0:KC * DMODEL // 2],
                                    wv_d[:, 0:KC * DMODEL // 2])
                nc.gpsimd.dma_start(wv_t[:, KC * DMODEL // 2:],
                                    wv_d[:, KC * DMODEL // 2:])
                # remaining W_q/W_k pair blocks, in consumption order
                for m in range(2, KC):
                    load_w(m)

                xhalf = [xa_t, xb_t]

                def qk_chunk(mm, n):
                    if n == 0:
                        qkt[mm] = qkpool.tile(
                            [128, NSEQ], F16, tag="qkt", name=f"qkt{mm}")
                    ps = mps.tile([128, 512], F32, tag="mps", name="ps_qk")
                    off = 0 if mm < KC else 128
                    w3 = w_t[mm % KC].rearrange("p (k c) -> p k c", c=256)
                    for k in range(KC):
                        nc.tensor.matmul(
                            ps[:],
                            lhsT=w3[:, k, off:off + 128],
                            rhs=xhalf[n][:, k * 512:(k + 1) * 512],
                            start=(k == 0), stop=(k == KC - 1),
                        )
                    if with_bias:
                        nc.vector.tensor_scalar_add(
                            qkt[mm][:, n * 512:(n + 1) * 512], ps[:],
                            b_qk[:, mm:mm + 1],
                        )
                    else:
                        nc.vector.tensor_copy(
                            qkt[mm][:, n * 512:(n + 1) * 512], ps[:])

                def v_chunk(mi, n0, nw):
                    ps = mps.tile([128, 512], F32, tag="mps", name="ps_v")
                    xh = xhalf[mi // 4]
                    c0 = (mi % 4) * 128
                    for k in range(KC):
                        nc.tensor.matmul(
                            ps[:, :nw],
                            lhsT=xh[:, k * 512 + c0:k * 512 + c0 + 128],
                            rhs=wv_t[:, k * DMODEL + n0:k * DMODEL + n0 + nw],
                            start=(k == 0), stop=(with_bias is False and k == KC - 1),
                        )
                    if with_bias:
                        nc.tensor.matmul(
                            ps[:, :nw], lhsT=ones1[:, :],
                            rhs=b_v[:, n0:n0 + nw], start=False, stop=True,
                        )
                    nh = nw // DH
                    h0 = n0 // DH
                    src = ps[:, :nw].rearrange("p (h c) -> p h c", c=DH)
                    dst3 = v_ext[mi].rearrange("p (h c) -> p h c", c=VB)
                    nc.vector.tensor_copy(dst3[:, h0:h0 + nh, 0:DH], src)

                for mi in range(MI):
                    d3 = v_ext[mi].rearrange("p (h c) -> p h c", c=VB)
                    nc.vector.memset(d3[:, :, DH:DH + 1], 1.0)

                pvq = deque()  # deferred PV i-steps: (head, i, E tiles)
                # 4 PV accumulation groups per PSUM tile (128-elem slots in a
                # [128, 512] bank) so the pv chain never stalls on buf rotation
                pvstate = {"tile": None, "slot": 0}

                def pv_epilogue(h, i, pv, tail):
                    # all-DVE even in the tail: a ScalarE hand-off adds a
                    # cross-engine semaphore hop per step, costly at the
                    # half-clock the HAM applies to the drain
                    r = stpool.tile([128, 1], F32, tag="r", name="r")
                    nc.vector.reciprocal(r[:], pv[:, DH:DH + 1])
                    u = stpool.tile([128, DH], F32, tag="u", name="u")
                    nc.vector.tensor_scalar(
                        u[:], pv[:, 0:DH], r[:], None, op0=MUL)
                    nc.sync.dma_start(out_d[h, i], u[:])

                def pv_matmuls(h, i, E0, E1, pv, js, start, stop, newfmt):
                    # old (pair 0): slab = [headA qn | headB qn] per q-half n
                    # new (pairs 1+): slab = one head's full 1024-query row
                    if newfmt:
                        E = E0 if h % 2 == 0 else E1
                        c0 = i * 128
                    else:
                        E = E0 if i < 4 else E1
                        c0 = 512 * (h % 2) + (i % 4) * 128
                    for j in js:
                        nc.tensor.matmul(
                            pv,
                            lhsT=E[j][:, c0:c0 + 128],
                            rhs=v_ext[j][:, h * VB:(h + 1) * VB],
                            start=(start and j == js[0]),
                            stop=(stop and j == js[-1]),
                        )

                def pv_slot():
                    if pvstate["slot"] == 0:
                        pvstate["tile"] = mps.tile(
                            [128, 512], F32, tag="mps", name="pv4")
                    s0 = pvstate["slot"] * 128
                    pvstate["slot"] = (pvstate["slot"] + 1) % 4
                    return pvstate["tile"][:, s0:s0 + VB]

                def pv_step(h, i, E0, E1, newfmt, tail=False):
                    pv = pv_slot()
                    pv_matmuls(h, i, E0, E1, pv, list(range(MI)), True, True,
                               newfmt)
                    pv_epilogue(h, i, pv, tail)

                # prologue: only the n=0 halves — exp(ps0) starts after two
                # chunks; the n=1 halves compute as j=0/1 fillers
                for mm in (0, KC):
                    qk_chunk(mm, 0)

                # heads 0-7 (cols 0:512) first: pair-0's PV at pm1 j>=1 needs
                # them; the head-8-11 chunks aren't read before pair-4's PV
                vq = deque([(mi, 0, 512) for mi in range(MI)]
                           + [(mi, 512, 256) for mi in range(MI)])

                LASTPM = H // 2 - 1
                for pm in range(H // 2):
                    hA, hB = 2 * pm, 2 * pm + 1
                    q_t, k_t = qkt[pm], qkt[KC + pm]
                    EA, EB = [], []
                    ch_tiles = []

                    def chain_mms(cs, js):
                        # j-partial PV accumulation for the LAST pair: chain
                        # c = [hA i0-3 | hA i4-7 | hB i0-3 | hB i4-7], one
                        # [128,65] slot per chain, 4 per mix-psum tile.
                        # NO start=True anywhere: a start's has_written clear
                        # is bank-wide, so interleaved chains in one bank
                        # would wipe each other's partial sums -- the tiles
                        # are zeroed by memset instead and every matmul
                        # accumulates (or overwrites fresh elements).
                        for jp in js:
                            for c in cs:
                                h = hA if c < 8 else hB
                                i = c % 8
                                E = EA if c < 8 else EB  # per-head slabs
                                s0 = (c % 4) * 128
                                nc.tensor.matmul(
                                    ch_tiles[c // 4][:, s0:s0 + VB],
                                    lhsT=E[jp][:, i * 128:(i + 1) * 128],
                                    rhs=v_ext[jp][:, h * VB:(h + 1) * VB],
                                    start=False, stop=(jp == MI - 1),
                                )

                    nxt = []
                    if pm + 1 < H // 2:
                        nxt = [(pm + 1, 0), (pm + 1, 1),
                               (KC + pm + 1, 0), (KC + pm + 1, 1)]
                    for j in range(MI):
                        # pm0: A and B share each slab ([A qn | B qn] halves,
                        # row-tiled concurrent MMs); slab ps0 only needs the
                        # n=0 q/k halves so the exp stream starts early
                        def s_half(jj, n, elist):
                            psn = sps.tile(
                                [128, NSEQ], F32, tag="sps", name=f"ps{n}")
                            nc.tensor.matmul(
                                psn[:, 0:512],
                                lhsT=k_t[0:64, jj * 128:(jj + 1) * 128],
                                rhs=q_t[0:64, n * 512:(n + 1) * 512],
                                start=True, stop=True, tile_position=(0, 0),
                            )
                            nc.tensor.matmul(
                                psn[:, 512:1024],
                                lhsT=k_t[64:128, jj * 128:(jj + 1) * 128],
                                rhs=q_t[64:128, n * 512:(n + 1) * 512],
                                start=True, stop=True, tile_position=(64, 0),
                            )
                            e = epool.tile(
                                [128, NSEQ], BF16, tag="e", name=f"e{n}")
                            nc.scalar.activation(e[:], psn[:], EXP)
                            elist.append(e)

                        # pm1+: slab = one head's full 1024-query row, built
                        # by two N=512 matmuls sharing ONE stationary (walrus
                        # rejects F=1024 into PSUM). Head A uses only row
                        # group 0:64 and head B only 64:128, so the two
                        # heads' LDW+MM chains fully overlap -- unlike the
                        # [A|B]-half slabs, whose back-to-back slabs
                        # serialize on same-row-group LDWEIGHTS.
                        def s_full(jj, x, elist):
                            psn = sps.tile(
                                [128, NSEQ], F32, tag="sps", name=f"psf{x}")
                            r0 = 64 * x
                            w = k_t[r0:r0 + 64, jj * 128:(jj + 1) * 128]
                            for n in range(2):
                                nc.tensor.matmul(
                                    psn[:, n * 512:(n + 1) * 512], lhsT=w,
                                    rhs=q_t[r0:r0 + 64,
                                            n * 512:(n + 1) * 512],
                                    start=True, stop=True,
                                    tile_position=(r0, 0),
                                )
                            e = epool.tile(
                                [128, NSEQ], BF16, tag="e", name=f"ef{x}")
                            nc.scalar.activation(e[:], psn[:], EXP)
                            elist.append(e)

                        if pm > 0:
                            s_full(j, 0, EA)
                            s_full(j, 1, EB)
                            defer = False
                        else:
                            s_half(j, 0, EA)
                            defer = j < 4
                            if not defer:
                                if j == 4:
                                    for jj in range(4):
                                        s_half(jj, 1, EB)
                                s_half(j, 1, EB)
                        # fill work after the S pair: lower scheduler priority,
                        # so it runs only while S matmuls are stalled
                        if defer and j >= 2:
                            # this pair's n=1 q/k halves (xb lands mid-entry)
                            qk_chunk((0, KC)[j - 2], 1)
                        if vq and ((pm == 0 and j >= 4) or pm == 1):
                            for _ in range(2):
                                if vq:
                                    v_chunk(*vq.popleft())
                        if nxt and j % 2 == 0 and (pm > 0 or j >= 2):
                            qk_chunk(*nxt.pop(0))
                            if pm == 0 and j == 6 and nxt:
                                qk_chunk(*nxt.pop(0))
                        # none at j=0/j=7: fillers there would sit between the
                        # previous pair's last S and this pair's first S in
                        # the in-order PE queue, stalling the exp stream
                        # (pm5 front-loads pair-4's steps at j=1..4 so the mix
                        # psum frees up for its own j-partial chains)
                        cnts = ((0, 2, 2, 3, 3, 3, 3, 0) if pm < LASTPM
                                else (0, 4, 4, 4, 4, 0, 0, 0))
                        for _ in range(cnts[j]):
                            if pvq:
                                pv_step(*pvq.popleft())
                        if pm == LASTPM:
                            # the last pair's own PV accumulates j-partially
                            # under its exp stream; only the j'=7 matmuls and
                            # the epilogue remain after the final exp
                            if j == 5:
                                for t in range(2):
                                    ch = mps.tile([128, 512], F32, tag="mps",
                                                  name=f"pvch{t}")
                                    nc.vector.memset(ch[:], 0.0)
                                    ch_tiles.append(ch)
                                chain_mms(range(0, 8), (0, 1, 2))
                            elif j == 6:
                                for t in range(2, 4):
                                    ch = mps.tile([128, 512], F32, tag="mps",
                                                  name=f"pvch{t}")
                                    nc.vector.memset(ch[:], 0.0)
                                    ch_tiles.append(ch)
                                chain_mms(range(8, 16), (0, 1, 2))
                                chain_mms(range(0, 8), (3, 4))
                            elif j == 7:
                                chain_mms(range(0, 8), (5, 6))
                                chain_mms(range(8, 16), (3, 4, 5, 6))
                                # j'=7: A-half consumers first (their exp
                                # completes one slab earlier), then B-half
                                chain_mms((0, 1, 2, 3, 8, 9, 10, 11), (7,))
                                chain_mms((4, 5, 6, 7, 12, 13, 14, 15), (7,))
                    if pm < LASTPM:
                        pvq.extend((hA, i, EA, EB, pm >= 1) for i in range(MI))
                        pvq.extend((hB, i, EA, EB, pm >= 1) for i in range(MI))
                # batched epilogue of the last pair's chain tiles: ONE strided
                # reciprocal per tile (4 denominators) -- the post-exp tail is
                # bound by serialized DVE ops
                assert not pvq
                tail_tiles = [
                    (ch_tiles[t],
                     [(H - 2 + t // 2, (t % 2) * 4 + k) for k in range(4)])
                    for t in range(4)
                ]
                for tile4, steps in tail_tiles:
                    t3 = tile4.rearrange("p (s c) -> p s c", c=128)
                    r4 = stpool.tile([128, 4], F32, tag="r4", name="r4")
                    nc.vector.reciprocal(
                        r4.rearrange("p (s c) -> p s c", c=1)[:, :len(steps)],
                        t3[:, :len(steps), DH:DH + 1])
                    # scales split across DVE and the otherwise-idle ScalarE
                    # (they run concurrently); one DMA per tile - the 4 steps
                    # are same-head consecutive-i, contiguous in DRAM
                    u4 = stpool.tile([128, 4 * DH], F32, tag="u4", name="u4")
                    for k, (h, i) in enumerate(steps):
                        if k % 2 == 0:
                            nc.vector.tensor_scalar(
                                u4[:, k * DH:(k + 1) * DH], t3[:, k, 0:DH],
                                r4[:, k:k + 1], None, op0=MUL)
                        else:
                            nc.scalar.activation(
                                u4[:, k * DH:(k + 1) * DH], t3[:, k, 0:DH],
                                mybir.ActivationFunctionType.Copy,
                                scale=r4[:, k:k + 1])
                    h0, i0 = steps[0]
                    nc.sync.dma_start(
                        out_d[h0, i0:i0 + len(steps)].rearrange(
                            "s p c -> p s c"),
                        u4.rearrange("p (s c) -> p s c", c=DH)[:, :len(steps)])

    nc.compile()
    _NC_CACHE[key] = nc
    return nc


def make_in_maps(x, W_qkv, b_qkv):
    x = np.asarray(x, dtype=np.float32)
    W_qkv = np.asarray(W_qkv, dtype=np.float32)
    b_qkv = np.asarray(b_qkv, dtype=np.float32)
    xT = x.transpose(0, 2, 1)                                # (B, 768, 1024)
    # xa per-chunk (low first-use latency); xb grouped in halves of 3 chunks
    # packed per-partition-contiguous for 3KB DMA descriptors
    xa = np.ascontiguousarray(
        xT[:, :, 0:512].reshape(N_CORES, KC, 128, 512)).astype(np.float16)
    xb = np.ascontiguousarray(
        xT[:, :, 512:1024].reshape(N_CORES, 2, 3, 128, 512)
        .transpose(0, 1, 3, 2, 4).reshape(N_CORES, 2, 128, 1536)
        .astype(np.float16))
    # wp[pm] = [128 part, KC, 256] with q-chunk cols then k-chunk cols
    wr = W_qkv.reshape(KC, 128, C3)
    blocks = []
    for pm in range(KC):
        qp = wr[:, :, pm * 128:(pm + 1) * 128]               # (KC, 128, 128)
        kp = wr[:, :, DMODEL + pm * 128:DMODEL + (pm + 1) * 128]
        blocks.append(np.concatenate([qp, kp], axis=2)       # (KC, 128, 256)
                      .transpose(1, 0, 2))                   # (128, KC, 256)
    wp = np.ascontiguousarray(
        np.stack(blocks).reshape(KC, 128, KC * 256)).astype(np.float16)
    wv = np.ascontiguousarray(
        wr[:, :, 2 * DMODEL:C3].transpose(1, 0, 2)           # (128, KC, 768)
        .reshape(128, KC * DMODEL)).astype(np.float16)
    b_qk = np.ascontiguousarray(
        b_qkv[:2 * DMODEL].reshape(2 * KC, 128).T)           # (128, 12)
    b_v = np.ascontiguousarray(b_qkv[2 * DMODEL:].reshape(1, DMODEL))
    ones_in = np.ones((1, 128), dtype=np.float32)
    return [
        {"xa": xa[c], "xb": xb[c], "wp": wp, "wv": wv,
         "b_qk": b_qk, "b_v": b_v, "ones_in": ones_in}
        for c in range(N_CORES)
    ]


def run(in_maps, trace=False, trace_cores=None, with_bias=True):
    nc = build_nc(with_bias=with_bias)
    try:
        return run_bass_kernel_spmd(
            nc, in_maps, list(range(N_CORES)),
            trace=trace, trace_cores=trace_cores,
        )
    except Exception:
        # transient NRT_EXEC_UNIT_UNRECOVERABLE has been observed after
        # profiled runs; one retry after a pause usually recovers
        time.sleep(20)
        return run_bass_kernel_spmd(
            nc, in_maps, list(range(N_CORES)),
            trace=trace, trace_cores=trace_cores,
        )


def kernel(x, W_qkv, b_qkv):
    with_bias = bool(np.any(np.asarray(b_qkv)))
    res = run(make_in_maps(x, W_qkv, b_qkv), with_bias=with_bias)
    # device writes (H, MI, 128, DH); reassemble to (NSEQ, DMODEL)
    outs = [
        res.results[c]["out"].transpose(1, 2, 0, 3).reshape(NSEQ, DMODEL)
        for c in range(N_CORES)
    ]
    return np.stack(outs).astype(np.float32)



# revision 29
# speedup vs baseline: 1.0022x; 1.0022x over previous
"""Multi-head attention (B=8, N=1024, D=768, H=12) on 8 TRN2 NeuronCores.

Sharding: data-parallel over batch - core b computes batch element b.

Host-prepped per-core inputs (QK-path operands shipped as fp16 - the
e5m10 mantissa keeps the end-to-end error ~6.4e-3 vs the 2e-2 budget,
halves DMA bytes, and 2-byte LDWEIGHTS double-buffer in the PE, unlike
fp32 weight loads which expose ~100-190ns per matmul):
    xa    (6, 128, 512)   x[b]^T contraction chunks, columns 0:512
    xb    (2, 128, 1536)  columns 512:1024, packed per-partition-
                          contiguous in halves for 3KB DMA descriptors
    wp    (6, 128, 1536)  W_q/W_k packed per head pair ([q cols | k cols]
                          per k-chunk; one contiguous DMA per pair)
    wv    (128, 4608)     W_v chunks packed per-partition-contiguous
    b_qk (128, 12), b_v (1, 768), ones_in (1, 128)
  output: out (12, 8, 128, 64) f32, host-reassembled to (1024, 768)

Entry: the first-S critical set (wp0 + xa, 1.18MB) is spread evenly
over the three DMA-capable queues (sync/scalar/gpsimd, ~21GB/s each).

Single fully-interleaved pipeline; ScalarE's exp stream (96 x [128,1024]
at ~1.11us each) is the near-critical resource next to TensorE:
  - qkT chunks ((x @ W_qk)^T -> fp16, recycled 8-buf pool) are produced
    one head pair ahead; the first pair computes only the n=0 column
    halves up front so the first exp fires ~7us earlier, with the n=1
    halves and their retroactive S slabs emitted as j=0..2 fillers.
  - v = x @ W_v (+ b_v via a ones-row matmul, skipped when bias is zero)
    is stored per head as bf16 [v 64 | ones 1] blocks (the ones column
    yields the softmax denominator inside the PV matmul).
  - S^T[j,i] = k @ q^T per head pair via row-tiled K=64 fp16 matmuls; the
    two heads share each [128,1024] PSUM slab (A left bank, B right) so
    one exp releases both heads' next matmuls and the pair runs
    concurrently on PE row groups 0:64 / 64:128.
  - E = exp(S^T) -> bf16 on ScalarE straight from PSUM (bf16, not fp16:
    exp of +-40 logits overflows fp16). No max subtraction: softmax is
    shift-invariant and the logits stay in fp32/bf16 exp range.
  - PV: psum[i-block, 0:65] = sum_j E^T[j] @ v_block; pv accumulation
    groups pack 4-per-PSUM-bank (128-elem slots of one [128,512] tile)
    so the pv chain never stalls on pool rotation. PV i-steps of the
    previous pair and next pair's qkT chunks fill PE slack between S
    j-steps - but never at j=0/j=7, where they would sit between
    adjacent pairs' S matmuls in the in-order PE queue and stall the
    exp stream across the pair boundary.
  - epilogue per (head, i): out = pv * recip(denom), all-DVE (a ScalarE
    hand-off would add a cross-engine sem hop per step, costly in the
    half-clock tail), then one 32KB DMA per (head, i-block) into
    out (H, MI, 128, 64); the host reassembles to (1024, 768) for free.
  - PSUM: 2 S-slab bufs (4 banks) ping-pong against the exp stream;
    4 shared mix bufs (4 banks) decouple the qk/v/pv psum rotations
    (with only 2, a qk chunk or pv chain regularly stalled ~1us waiting
    a pv tile's DVE drain, surfacing as exp-stream gaps at pair
    boundaries).

A dummy exp at kernel start pulls the ~1.4us ACT table load into the
idle entry phase.

The tail (the final pair's 16 PV steps, which cannot overlap their own
exps and always run at the HAM's half clock) fills all 16 PSUM groups
first - they fit the 4-buf mix pool exactly - then drains with one
strided reciprocal per 4-slot tile, minimizing serialized DVE ops.

Measured: ~159us HW exec per core (all 8 cores run the same program),
max-abs relative error 6.4e-3 vs the fp32 reference. Note the device
has two clock states; slow-state runs measure ~20% longer with ScalarE
ops uniformly slower (exp 1.34us vs 1.11us) - compare kernels only
across same-state runs (exp avg duration in the trace identifies the
state).
"""

import time
from collections import deque

import numpy as np

import concourse.mybir as mybir
import concourse.tile as tile
from concourse import bacc
from concourse.bass_utils import run_bass_kernel_spmd

N_CORES = 8
NSEQ = 1024
DMODEL = 768
H = 12
DH = 64
C3 = 3 * DMODEL
KC = DMODEL // 128   # 6 contraction chunks
MI = NSEQ // 128     # 8 sequence chunks
VB = DH + 1          # 65: per-head v block [v bf16 64 | ones 1]

F32 = mybir.dt.float32
F32R = mybir.dt.float32r
F16 = mybir.dt.float16
BF16 = mybir.dt.bfloat16
EXP = mybir.ActivationFunctionType.Exp
MUL = mybir.AluOpType.mult
ADD = mybir.AluOpType.add

_NC_CACHE = {}


def build_nc(with_bias=True):
    key = ("nc", with_bias)
    if key in _NC_CACHE:
        return _NC_CACHE[key]
    nc = bacc.Bacc("TRN2", target_bir_lowering=False, debug=False)
    xa_d = nc.dram_tensor("xa", [KC, 128, 512], F16, kind="ExternalInput")
    xb_d = nc.dram_tensor("xb", [2, 128, 3 * 512], F16, kind="ExternalInput")
    wp_d = nc.dram_tensor("wp", [KC, 128, KC * 256], F16, kind="ExternalInput")
    wv_d = nc.dram_tensor("wv", [128, KC * DMODEL], F16, kind="ExternalInput")
    bqk_d = nc.dram_tensor("b_qk", [128, 2 * KC], F32, kind="ExternalInput")
    bv_d = nc.dram_tensor("b_v", [1, DMODEL], F32R, kind="ExternalInput")
    ones_d = nc.dram_tensor("ones_in", [1, 128], F32R, kind="ExternalInput")
    out_d = nc.dram_tensor("out", [H, MI, 128, DH], F32, kind="ExternalOutput")

    with tile.TileContext(nc) as tc:
        with (
            tc.tile_pool(name="const", bufs=1) as cpool,
            tc.tile_pool(name="main", bufs=1) as mpool,
            tc.tile_pool(name="stage", bufs=12) as stpool,
            tc.tile_pool(name="e", bufs=34) as epool,
            tc.tile_pool(name="wt", bufs=3) as wpool,
            tc.tile_pool(name="qkt", bufs=8) as qkpool,
            tc.tile_pool(name="s_ps", bufs=2, space="PSUM") as sps,
            tc.tile_pool(name="mix_ps", bufs=4, space="PSUM") as mps,
        ):
            b_qk = cpool.tile([128, 2 * KC], F32, tag="bqk")
            nc.sync.dma_start(b_qk[:], bqk_d[:])
            warm = cpool.tile([128, 1], F32, tag="warm")
            nc.scalar.activation(warm[:], b_qk[:, 0:1], EXP)
            # PE warm-up: ~9us of dummy matmuls during the input-DMA wait
            # pulls the HAM to K=8/8 before the first real qk chunk, halving
            # the entry matmuls' duration (cold MMs run at 1.2 GHz).
            dummy = cpool.tile([128, 512], F16, tag="dummy")
            nc.vector.memset(dummy[:], 0.0)
            dps = mps.tile([128, 512], F32, tag="mps", name="ps_warm")
            NWARM = 30
            for r in range(NWARM):
                nc.tensor.matmul(
                    dps[:], lhsT=dummy[:, 0:128], rhs=dummy[:],
                    start=(r == 0), stop=(r == NWARM - 1),
                )

            # persistent activations
            v_ext = [mpool.tile([128, H * VB], BF16, tag=f"vx{j}", name=f"vx{j}")
                     for j in range(MI)]
            # x^T halves, k-chunks packed as column blocks of one tile
            xa_t = mpool.tile([128, KC * 512], F16, tag="xat", name="xat")
            xb_t = mpool.tile([128, KC * 512], F16, tag="xbt", name="xbt")

            # W_q/W_k packed per head pair: tile[:, k, 0:128] = q chunk cols,
            # tile[:, k, 128:256] = k chunk cols. One contiguous DMA per pair.
            w_t = {}

            def load_w(pm):
                t = wpool.tile([128, KC * 256], F16, tag="w", name=f"wp{pm}")
                nc.sync.dma_start(t[:], wp_d[pm])
                w_t[pm] = t

            # qkT chunks from a recycled pool (live: current + next pair)
            qkt = {}

            # inputs split across the three DMA-capable queues (sync/scalar/
            # gpsimd); the first-S critical set (wp0 + xa) leads each queue
            # first-S critical set lands in k-consumption order: wp0 in
            # three k-pair blocks on sync, xa as 2+1 chunks per side queue,
            # so the entry qk matmuls pipeline behind the transfers
            t0w = wpool.tile([128, KC * 256], F16, tag="w", name="wp0")
            nc.sync.dma_start(t0w[:, 0:512], wp_d[0][:, 0:512])
            nc.sync.dma_start(t0w[:, 512:1024], wp_d[0][:, 512:1024])
            nc.sync.dma_start(t0w[:, 1024:1536], wp_d[0][:, 1024:1536])
            w_t[0] = t0w
            xa3 = xa_t.rearrange("p (k c) -> p k c", c=512)
            nc.scalar.dma_start(xa3[:, 0:2], xa_d[0:2].rearrange("k p c -> p k c"))
            nc.gpsimd.dma_start(xa3[:, 3:5], xa_d[3:5].rearrange("k p c -> p k c"))
            nc.scalar.dma_start(xa_t[:, 1024:1536], xa_d[2])
            nc.gpsimd.dma_start(xa_t[:, 2560:3072], xa_d[5])

            with tc.tile_pool(name="wv", bufs=1) as wvpool:
                load_w(1)  # sync: wp1 ahead of the small const DMAs
                b_v = cpool.tile([1, DMODEL], F32R, tag="bv")
                nc.sync.dma_start(b_v[:], bv_d[:])
                ones1 = cpool.tile([1, 128], F32R, tag="ones")
                nc.sync.dma_start(ones1[:], ones_d[:])
                nc.scalar.dma_start(xb_t[:, 0:1536], xb_d[0])
                nc.gpsimd.dma_start(xb_t[:, 1536:3072], xb_d[1])
                wv_t = wvpool.tile([128, KC * DMODEL], F16, tag="wvt", name="wvt")
                nc.scalar.dma_start(wv_t[:, 0:KC * DMODEL // 2],
                                    wv_d[:, 0:KC * DMODEL // 2])
                nc.gpsimd.dma_start(wv_t[:, KC * DMODEL // 2:],
                                    wv_d[:, KC * DMODEL // 2:])
                # remaining W_q/W_k pair blocks, in consumption order
                for m in range(2, KC):
                    load_w(m)

                xhalf = [xa_t, xb_t]

                def qk_chunk(mm, n):
                    if n == 0:
                        qkt[mm] = qkpool.tile(
                            [128, NSEQ], F16, tag="qkt", name=f"qkt{mm}")
                    ps = mps.tile([128, 512], F32, tag="mps", name="ps_qk")
                    off = 0 if mm < KC else 128
                    w3 = w_t[mm % KC].rearrange("p (k c) -> p k c", c=256)
                    for k in range(KC):
                        nc.tensor.matmul(
                            ps[:],
                            lhsT=w3[:, k, off:off + 128],
                            rhs=xhalf[n][:, k * 512:(k + 1) * 512],
                            start=(k == 0), stop=(k == KC - 1),
                        )
                    # medium priority: the cast gates the NEXT pair's S
                    # stream; don't let it queue behind a batch of epilogues
                    with tc.high_priority(offset=10**5):
                        if with_bias:
                            nc.vector.tensor_scalar_add(
                                qkt[mm][:, n * 512:(n + 1) * 512], ps[:],
                                b_qk[:, mm:mm + 1],
                            )
                        else:
                            nc.vector.tensor_copy(
                                qkt[mm][:, n * 512:(n + 1) * 512], ps[:])

                def v_chunk(mi, n0, nw):
                    ps = mps.tile([128, 512], F32, tag="mps", name="ps_v")
                    xh = xhalf[mi // 4]
                    c0 = (mi % 4) * 128
                    for k in range(KC):
                        nc.tensor.matmul(
                            ps[:, :nw],
                            lhsT=xh[:, k * 512 + c0:k * 512 + c0 + 128],
                            rhs=wv_t[:, k * DMODEL + n0:k * DMODEL + n0 + nw],
                            start=(k == 0), stop=(with_bias is False and k == KC - 1),
                        )
                    if with_bias:
                        nc.tensor.matmul(
                            ps[:, :nw], lhsT=ones1[:, :],
                            rhs=b_v[:, n0:n0 + nw], start=False, stop=True,
                        )
                    nh = nw // DH
                    h0 = n0 // DH
                    src = ps[:, :nw].rearrange("p (h c) -> p h c", c=DH)
                    dst3 = v_ext[mi].rearrange("p (h c) -> p h c", c=VB)
                    nc.vector.tensor_copy(dst3[:, h0:h0 + nh, 0:DH], src)

                for mi in range(MI):
                    d3 = v_ext[mi].rearrange("p (h c) -> p h c", c=VB)
                    nc.vector.memset(d3[:, :, DH:DH + 1], 1.0)

                pvq = deque()  # deferred PV i-steps: (head, i, E tiles)
                # 4 PV accumulation groups per PSUM tile (128-elem slots in a
                # [128, 512] bank) so the pv chain never stalls on buf rotation
                pvstate = {"tile": None, "slot": 0}

                def pv_epilogue(h, i, pv, tail):
                    # all-DVE even in the tail: a ScalarE hand-off adds a
                    # cross-engine semaphore hop per step, costly at the
                    # half-clock the HAM applies to the drain
                    r = stpool.tile([128, 1], F32, tag="r", name="r")
                    nc.vector.reciprocal(r[:], pv[:, DH:DH + 1])
                    u = stpool.tile([128, DH], F32, tag="u", name="u")
                    nc.vector.tensor_scalar(
                        u[:], pv[:, 0:DH], r[:], None, op0=MUL)
                    nc.sync.dma_start(out_d[h, i], u[:])

                def pv_matmuls(h, i, E0, E1, pv, js, start, stop, newfmt):
                    # old (pair 0): slab = [headA qn | headB qn] per q-half n
                    # new (pairs 1+): slab = one head's full 1024-query row
                    if newfmt:
                        E = E0 if h % 2 == 0 else E1
                        c0 = i * 128
                    else:
                        E = E0 if i < 4 else E1
                        c0 = 512 * (h % 2) + (i % 4) * 128
                    for j in js:
                        nc.tensor.matmul(
                            pv,
                            lhsT=E[j][:, c0:c0 + 128],
                            rhs=v_ext[j][:, h * VB:(h + 1) * VB],
                            start=(start and j == js[0]),
                            stop=(stop and j == js[-1]),
                        )

                def pv_slot():
                    if pvstate["slot"] == 0:
                        pvstate["tile"] = mps.tile(
                            [128, 512], F32, tag="mps", name="pv4")
                    s0 = pvstate["slot"] * 128
                    pvstate["slot"] = (pvstate["slot"] + 1) % 4
                    return pvstate["tile"][:, s0:s0 + VB]

                def pv_step(h, i, E0, E1, newfmt, tail=False):
                    pv = pv_slot()
                    pv_matmuls(h, i, E0, E1, pv, list(range(MI)), True, True,
                               newfmt)
                    pv_epilogue(h, i, pv, tail)

                # prologue: only the n=0 halves — exp(ps0) starts after two
                # chunks; the n=1 halves compute as j=0/1 fillers
                for mm in (0, KC):
                    qk_chunk(mm, 0)

                # heads 0-7 (cols 0:512) first: pair-0's PV at pm1 j>=1 needs
                # them; the head-8-11 chunks aren't read before pair-4's PV
                vq = deque([(mi, 0, 512) for mi in range(MI)]
                           + [(mi, 512, 256) for mi in range(MI)])

                LASTPM = H // 2 - 1
                for pm in range(H // 2):
                    hA, hB = 2 * pm, 2 * pm + 1
                    q_t, k_t = qkt[pm], qkt[KC + pm]
                    EA, EB = [], []
                    ch_tiles = []

                    def chain_mms(cs, js):
                        # j-partial PV accumulation for the LAST pair: chain
                        # c = [hA i0-3 | hA i4-7 | hB i0-3 | hB i4-7], one
                        # [128,65] slot per chain, 4 per mix-psum tile.
                        # NO start=True anywhere: a start's has_written clear
                        # is bank-wide, so interleaved chains in one bank
                        # would wipe each other's partial sums -- the tiles
                        # are zeroed by memset instead and every matmul
                        # accumulates (or overwrites fresh elements).
                        for jp in js:
                            for c in cs:
                                h = hA if c < 8 else hB
                                i = c % 8
                                E = EA if c < 8 else EB  # per-head slabs
                                s0 = (c % 4) * 128
                                nc.tensor.matmul(
                                    ch_tiles[c // 4][:, s0:s0 + VB],
                                    lhsT=E[jp][:, i * 128:(i + 1) * 128],
                                    rhs=v_ext[jp][:, h * VB:(h + 1) * VB],
                                    start=False, stop=(jp == MI - 1),
                                )

                    def drain_chain_tile(t):
                        # ONE strided reciprocal per tile (4 denominators),
                        # scales split across DVE and ScalarE, one DMA per
                        # tile (4 same-head consecutive-i steps)
                        h0 = H - 2 + t // 2
                        i0 = (t % 2) * 4
                        t3 = ch_tiles[t].rearrange("p (s c) -> p s c", c=128)
                        r4 = stpool.tile([128, 4], F32, tag="r4", name="r4")
                        nc.vector.reciprocal(
                            r4.rearrange("p (s c) -> p s c", c=1)[:, :4],
                            t3[:, :4, DH:DH + 1])
                        u4 = stpool.tile([128, 4 * DH], F32, tag="u4",
                                         name="u4")
                        for k in range(4):
                            if k % 2 == 0:
                                nc.vector.tensor_scalar(
                                    u4[:, k * DH:(k + 1) * DH],
                                    t3[:, k, 0:DH],
                                    r4[:, k:k + 1], None, op0=MUL)
                            else:
                                nc.scalar.activation(
                                    u4[:, k * DH:(k + 1) * DH],
                                    t3[:, k, 0:DH],
                                    mybir.ActivationFunctionType.Copy,
                                    scale=r4[:, k:k + 1])
                        nc.sync.dma_start(
                            out_d[h0, i0:i0 + 4].rearrange("s p c -> p s c"),
                            u4.rearrange("p (s c) -> p s c", c=DH)[:, :4])

                    nxt = []
                    if pm + 1 < H // 2:
                        nxt = [(pm + 1, 0), (pm + 1, 1),
                               (KC + pm + 1, 0), (KC + pm + 1, 1)]
                    for j in range(MI):
                        # pm0: A and B share each slab ([A qn | B qn] halves,
                        # row-tiled concurrent MMs); slab ps0 only needs the
                        # n=0 q/k halves so the exp stream starts early
                        # the S matmuls + exps are the pace-setting stream:
                        # high priority so the scheduler never wedges filler
                        # matmuls between a slab's MMs or ahead of a ready S
                        def s_half(jj, n, elist):
                            psn = sps.tile(
                                [128, NSEQ], F32, tag="sps", name=f"ps{n}")
                            with tc.high_priority(offset=10**6):
                                nc.tensor.matmul(
                                    psn[:, 0:512],
                                    lhsT=k_t[0:64, jj * 128:(jj + 1) * 128],
                                    rhs=q_t[0:64, n * 512:(n + 1) * 512],
                                    start=True, stop=True,
                                    tile_position=(0, 0),
                                )
                                nc.tensor.matmul(
                                    psn[:, 512:1024],
                                    lhsT=k_t[64:128, jj * 128:(jj + 1) * 128],
                                    rhs=q_t[64:128, n * 512:(n + 1) * 512],
                                    start=True, stop=True,
                                    tile_position=(64, 0),
                                )
                                e = epool.tile(
                                    [128, NSEQ], BF16, tag="e", name=f"e{n}")
                                nc.scalar.activation(e[:], psn[:], EXP)
                            elist.append(e)

                        # pm1+: slab = one head's full 1024-query row, built
                        # by two N=512 matmuls sharing ONE stationary (walrus
                        # rejects F=1024 into PSUM). Head A uses only row
                        # group 0:64 and head B only 64:128, so the two
                        # heads' LDW+MM chains fully overlap -- unlike the
                        # [A|B]-half slabs, whose back-to-back slabs
                        # serialize on same-row-group LDWEIGHTS.
                        def s_full(jj, x, elist):
                            psn = sps.tile(
                                [128, NSEQ], F32, tag="sps", name=f"psf{x}")
                            r0 = 64 * x
                            w = k_t[r0:r0 + 64, jj * 128:(jj + 1) * 128]
                            with tc.high_priority(offset=10**6):
                                for n in range(2):
                                    nc.tensor.matmul(
                                        psn[:, n * 512:(n + 1) * 512], lhsT=w,
                                        rhs=q_t[r0:r0 + 64,
                                                n * 512:(n + 1) * 512],
                                        start=True, stop=True,
                                        tile_position=(r0, 0),
                                    )
                                e = epool.tile(
                                    [128, NSEQ], BF16, tag="e", name=f"ef{x}")
                                nc.scalar.activation(e[:], psn[:], EXP)
                            elist.append(e)

                        if pm > 0:
                            s_full(j, 0, EA)
                            s_full(j, 1, EB)
                            defer = False
                        else:
                            s_half(j, 0, EA)
                            defer = j < 4
                            if not defer:
                                if j == 4:
                                    for jj in range(4):
                                        s_half(jj, 1, EB)
                                s_half(j, 1, EB)
                        # fill work after the S pair: lower scheduler priority,
                        # so it runs only while S matmuls are stalled
                        if defer and j >= 2:
                            # this pair's n=1 q/k halves (xb lands mid-entry)
                            qk_chunk((0, KC)[j - 2], 1)
                        if vq and ((pm == 0 and j >= 4) or pm == 1):
                            for _ in range(2):
                                if vq:
                                    v_chunk(*vq.popleft())
                        if nxt and j % 2 == 0 and (pm > 0 or j >= 2):
                            qk_chunk(*nxt.pop(0))
                            if pm == 0 and j == 6 and nxt:
                                qk_chunk(*nxt.pop(0))
                        # none at j=0/j=7: fillers there would sit between the
                        # previous pair's last S and this pair's first S in
                        # the in-order PE queue, stalling the exp stream
                        # (pm5 front-loads pair-4's steps at j=1..4 so the mix
                        # psum frees up for its own j-partial chains)
                        cnts = ((0, 2, 2, 3, 3, 3, 3, 0) if pm < LASTPM
                                else (0, 4, 4, 4, 4, 0, 0, 0))
                        for _ in range(cnts[j]):
                            if pvq:
                                pv_step(*pvq.popleft())
                        if pm == LASTPM:
                            # the last pair's own PV accumulates j-partially
                            # under its exp stream; only the j'=7 matmuls and
                            # the epilogue remain after the final exp
                            if j == 5:
                                for t in range(2):
                                    ch = mps.tile([128, 512], F32, tag="mps",
                                                  name=f"pvch{t}")
                                    nc.vector.memset(ch[:], 0.0)
                                    ch_tiles.append(ch)
                                chain_mms(range(0, 8), (0, 1, 2))
                            elif j == 6:
                                for t in range(2, 4):
                                    ch = mps.tile([128, 512], F32, tag="mps",
                                                  name=f"pvch{t}")
                                    nc.vector.memset(ch[:], 0.0)
                                    ch_tiles.append(ch)
                                chain_mms(range(8, 16), (0, 1, 2))
                                chain_mms(range(0, 8), (3, 4))
                            elif j == 7:
                                chain_mms(range(0, 8), (5, 6))
                                chain_mms(range(8, 16), (3, 4, 5, 6))
                                # j'=7: head-A chains (c0-7, tiles 0-1) can
                                # finish off exp(EA[7]), one slab before the
                                # final exp(EB[7]) releases c8-15; drain each
                                # tile as its 4 chains complete
                                chain_mms(range(0, 8), (7,))
                                drain_chain_tile(0)
                                drain_chain_tile(1)
                                chain_mms(range(8, 16), (7,))
                                drain_chain_tile(2)
                                drain_chain_tile(3)
                    if pm < LASTPM:
                        pvq.extend((hA, i, EA, EB, pm >= 1) for i in range(MI))
                        pvq.extend((hB, i, EA, EB, pm >= 1) for i in range(MI))
                assert not pvq

    nc.compile()
    _NC_CACHE[key] = nc
    return nc


def make_in_maps(x, W_qkv, b_qkv):
    x = np.asarray(x, dtype=np.float32)
    W_qkv = np.asarray(W_qkv, dtype=np.float32)
    b_qkv = np.asarray(b_qkv, dtype=np.float32)
    xT = x.transpose(0, 2, 1)                                # (B, 768, 1024)
    # xa per-chunk (low first-use latency); xb grouped in halves of 3 chunks
    # packed per-partition-contiguous for 3KB DMA descriptors
    xa = np.ascontiguousarray(
        xT[:, :, 0:512].reshape(N_CORES, KC, 128, 512)).astype(np.float16)
    xb = np.ascontiguousarray(
        xT[:, :, 512:1024].reshape(N_CORES, 2, 3, 128, 512)
        .transpose(0, 1, 3, 2, 4).reshape(N_CORES, 2, 128, 1536)
        .astype(np.float16))
    # wp[pm] = [128 part, KC, 256] with q-chunk cols then k-chunk cols
    wr = W_qkv.reshape(KC, 128, C3)
    blocks = []
    for pm in range(KC):
        qp = wr[:, :, pm * 128:(pm + 1) * 128]               # (KC, 128, 128)
        kp = wr[:, :, DMODEL + pm * 128:DMODEL + (pm + 1) * 128]
        blocks.append(np.concatenate([qp, kp], axis=2)       # (KC, 128, 256)
                      .transpose(1, 0, 2))                   # (128, KC, 256)
    wp = np.ascontiguousarray(
        np.stack(blocks).reshape(KC, 128, KC * 256)).astype(np.float16)
    wv = np.ascontiguousarray(
        wr[:, :, 2 * DMODEL:C3].transpose(1, 0, 2)           # (128, KC, 768)
        .reshape(128, KC * DMODEL)).astype(np.float16)
    b_qk = np.ascontiguousarray(
        b_qkv[:2 * DMODEL].reshape(2 * KC, 128).T)           # (128, 12)
    b_v = np.ascontiguousarray(b_qkv[2 * DMODEL:].reshape(1, DMODEL))
    ones_in = np.ones((1, 128), dtype=np.float32)
    return [
        {"xa": xa[c], "xb": xb[c], "wp": wp, "wv": wv,
         "b_qk": b_qk, "b_v": b_v, "ones_in": ones_in}
        for c in range(N_CORES)
    ]


def run(in_maps, trace=False, trace_cores=None, with_bias=True):
    nc = build_nc(with_bias=with_bias)
    try:
        return run_bass_kernel_spmd(
            nc, in_maps, list(range(N_CORES)),
            trace=trace, trace_cores=trace_cores,
        )
    except Exception:
        # transient NRT_EXEC_UNIT_UNRECOVERABLE has been observed after
        # profiled runs; one retry after a pause usually recovers
        time.sleep(20)
        return run_bass_kernel_spmd(
            nc, in_maps, list(range(N_CORES)),
            trace=trace, trace_cores=trace_cores,
        )


def kernel(x, W_qkv, b_qkv):
    with_bias = bool(np.any(np.asarray(b_qkv)))
    res = run(make_in_maps(x, W_qkv, b_qkv), with_bias=with_bias)
    # device writes (H, MI, 128, DH); reassemble to (NSEQ, DMODEL)
    outs = [
        res.results[c]["out"].transpose(1, 2, 0, 3).reshape(NSEQ, DMODEL)
        for c in range(N_CORES)
    ]
    return np.stack(outs).astype(np.float32)

